# revision 50
# baseline (speedup 1.0000x reference)
"""Multi-head causal attention (B=2, S=2048, D=1024, H=16) on 8 TRN2 NeuronCores.

Sharding: batch*head parallel. Core c handles batch b = c//4 and the 4
heads h in [4*(c%4), 4*(c%4)+4). Each core computes its heads' Q/K/V
projections (column-parallel), causal softmax attention, and its partial
row-parallel output projection; the host sums the 4 partial outputs per
batch (the AllReduce of row-parallel tensor parallelism).

On-device layout: everything is kept "transposed" (feature-major) so
every matmul contracts along the partition dimension:
  scoresT[k,q] = K Q^T      (per head, 128-row k-tiles x 512-col q-tiles)
  P^T = exp(scoresT/8 + mask/8)   (additive -1e9 causal mask)
  outT[d,q]   = sum_k V[k,d] P^T[k,q]   (PSUM-accumulated over k-tiles)
  sums[q]     = sum_k P^T[k,q]          (ones-vector matmul, col-packed)
  y[q,e]     += sum_hd outT_norm[hd,q] * w_oT[hd,e]
Softmax skips the max-subtraction: scores ~ N(0,1) so exp never
overflows, and exp(-1e9/8) underflows to exactly 0 like the
reference's masked_fill(-1e9).

Schedule (v2, rebuilt from the measured baseline trace):
- All DRAM operands are HOST-pretiled so every input DMA moves >=4KB
  contiguous lines (engine descriptor-gen cost ~0.65us instead of
  2-7us for strided patterns).
- DMA priority order: wq, xq, wk, xk (the 9MB that gates the attention
  stream), then maskt/idbf/wv/xvt/consts/wo, round-robin over the
  sync/gpsimd/scalar queues. The Scalar queue carries only early
  ungated loads so nothing blocks the exp stream.
- Front: warm-up matmuls (PE clock-gate ramp), then Q projection
  (m0+m1 per e-tile, DMA-paced, 8 PSUM banks), drains chunked per
  512 cols across Vector/Scalar/GpSimd, then K projection the same
  way reusing the banks. First scores fire ~0.6us after the first
  K-m0 drain chunk.
- Stream: flat software-pipelined attention as in the baseline, with
  passes ALTERNATING head-pairs ((0,0),(1,0),(0,1),(1,1),...) so the
  row-parallel output projection (needs both pairs per q-block)
  spreads through the stream instead of piling at the end.
- ob accumulator drains ride GpSimd (idle during the stream); psy
  casts ride Vector; y DMAs GpSimd (tail ones alternate with Scalar,
  which is idle after the last exp).
- Tail: the last pass's normalize is interleaved per-128-col chunk
  with its output-projection matmuls, casts and DMAs so the post-exp
  tail is a pipeline, not a serial chain.
"""

import numpy as np

D_MODEL = 1024
N_HEADS = 16
D_K = 64
B, S = 2, 2048
N_CORES = 8
HPC = 4            # heads per core
KT = S // 128      # 16 k-tiles
QT = S // 512      # 4 q-tiles
ET = D_MODEL // 128  # 8 e-tiles (contraction tiles for projections)

_PROG_CACHE = {}


def _build_program():
    import concourse.bacc as bacc_mod
    import concourse.mybir as mybir
    import concourse.tile as tile

    f32 = mybir.dt.float32
    f32r = mybir.dt.float32r
    bf16 = mybir.dt.bfloat16
    Exp = mybir.ActivationFunctionType.Exp

    nc = bacc_mod.Bacc(
        "TRN2", target_bir_lowering=False, debug=False, num_devices=N_CORES
    )

    xq = nc.dram_tensor("xq", [D_MODEL, S], bf16, kind="ExternalInput").ap()
    xk = nc.dram_tensor("xk", [D_MODEL, S], bf16, kind="ExternalInput").ap()
    xvt = nc.dram_tensor("xvt", [128, 4 * ET * 512], bf16, kind="ExternalInput").ap()
    wq = nc.dram_tensor("wq", [128, ET * 256], bf16, kind="ExternalInput").ap()
    wk = nc.dram_tensor("wk", [128, ET * 256], bf16, kind="ExternalInput").ap()
    wv = nc.dram_tensor("wv", [128, ET * 256], bf16, kind="ExternalInput").ap()
    wo = nc.dram_tensor("wo", [256, D_MODEL], bf16, kind="ExternalInput").ap()
    maskt = nc.dram_tensor("maskt", [128, 1024], bf16, kind="ExternalInput").ap()
    idbf = nc.dram_tensor("idbf", [128, 132], bf16, kind="ExternalInput").ap()
    consts = nc.dram_tensor("consts", [128, 193], f32r, kind="ExternalInput").ap()
    y = nc.dram_tensor("y", [S, D_MODEL], bf16, kind="ExternalOutput").ap()

    with (
        tile.TileContext(nc) as tc,
        nc.allow_low_precision("bf16 attention"),
        tc.tile_pool(name="persist", bufs=1) as pp,
    ):
        # ---- persistent SBUF tiles ----
        def persist(shape, dtype, name):
            return pp.tile(shape, dtype, name=name, tag=name)

        wq_sb = persist([128, ET * 256], bf16, "wq_sb")
        wk_sb = persist([128, ET * 256], bf16, "wk_sb")
        wv_sb = persist([128, ET * 256], bf16, "wv_sb")
        wo_sb = [persist([128, D_MODEL], bf16, f"wo_sb{p}") for p in range(2)]
        maskt_sb = persist([128, 1024], bf16, "maskt_sb")
        idbf_sb = persist([128, 132], bf16, "idbf_sb")
        consts_sb = persist([128, 193], f32r, "consts_sb")
        qt_sb = [persist([128, S], bf16, f"qt_sb{p}") for p in range(2)]
        kt_sb = [persist([128, S], bf16, f"kt_sb{p}") for p in range(2)]
        # 128 cols per head (64 V + 1 ones + 63 zeros): the attnV
        # stationary is then exactly 128 weight-rows (FWL trigger).
        v_sb = [persist([128, 512], bf16, f"v_sb{i}") for i in range(KT)]
        outt_sb = [persist([128, S], bf16, f"outt_sb{p}") for p in range(2)]

        # ---- PE warm-up ----
        # The PE HAM clock gate drops to K=4 half-clock after any multi-us
        # PE idle and needs ~3.4us of gapless activity to return to K=8.
        # Dummy matmuls on a memset tile cover the ramp until the first
        # projection e-tile lands (~9.5us: preamble ~7us + wq + xq chunk).
        warm_src = persist([128, 640], bf16, "warm_src")
        nc.vector.memset(warm_src[:], 0.0)
        for i in range(KT):
            nc.vector.memset(v_sb[i][:], 0.0)
        with tc.tile_pool(name="psW", bufs=1, space="PSUM") as psW:
            wt = psW.tile([128, 512], f32, name="warm", tag="warm")
            for w in range(9):
                nc.tensor.matmul(
                    wt[:], warm_src[:, 0:128], warm_src[:, 128:640],
                    start=True, stop=True,
                )

        # ---- DMA issue: everything up front, in priority order ----
        # Three queues (sync HWDGE, gpsimd SWDGE, scalar HWDGE) round-
        # robin.  The stream gate is wq+xq+wk+xk (9MB ~ 25us at HBM BW);
        # everything else follows.
        # all 16 x e-tiles live at once: no WAR gating on any input DMA
        # issue, so the three queues stream the 9MB gate continuously
        xvkp_ctx = tc.tile_pool(name="xvk", bufs=4)
        xvkp = xvkp_ctx.__enter__()
        xep_ctx = tc.tile_pool(name="xe", bufs=16, side="right")
        xep = xep_ctx.__enter__()

        SYNC, GP, SC = nc.sync, nc.gpsimd, nc.scalar
        vdma_tiles = [
            xvkp.tile([128, ET * 512], bf16, name=f"xvk_{b}", tag="xvk")
            for b in range(4)
        ]
        xe_tiles = {}
        for ti in range(2):
            for e in range(ET):
                xe_tiles[(ti, e)] = xep.tile(
                    [128, S], bf16, name=f"xe_{ti}_{e}", tag="xe"
                )

        # The tile scheduler reorders same-engine DMA issues; wait-hints
        # (scheduler-time lower bounds) keep everything that is not the
        # stream gate (wq+xq+wk+xk) behind it in HBM-bandwidth order.
        # wq + the first e-tile ride Sync (HW-DGE) back-to-back so the
        # first projection matmul can start ~11us.
        # wq in halves + first xq e-tile in 4 chunks, spread across the
        # queues so the first matmul starts ~10us (one queue alone is
        # only ~110GB/s and a monolithic wq would gate Q-e0 ~5us)
        SYNC.dma_start(out=wq_sb[:, 0:1024], in_=wq[:, 0:1024])
        GP.dma_start(out=wq_sb[:, 1024:2048], in_=wq[:, 1024:2048])
        for c in range(4):
            (SC, SYNC, GP, SC)[c].dma_start(
                out=xe_tiles[(0, 0)][:, c * 512 : (c + 1) * 512],
                in_=xq[0:128, c * 512 : (c + 1) * 512],
            )
        xq_engs = (SC, SYNC, GP, SC, SYNC, GP, SC)
        for e in range(1, ET):
            xq_engs[e - 1].dma_start(
                out=xe_tiles[(0, e)][:], in_=xq[e * 128 : (e + 1) * 128, :]
            )
        SYNC.dma_start(out=wk_sb[:], in_=wk[:])
        xk_engs = (SC, SYNC, SC, GP, SYNC, SC, SYNC, GP)
        for e in range(ET):
            xk_engs[e].dma_start(
                out=xe_tiles[(1, e)][:], in_=xk[e * 128 : (e + 1) * 128, :]
            )
        # scheduler sim-time maps ~0.21x to hardware time here (measured:
        # a 0.1ms hint lands ~21us real), so hints are ~4.8x the intended
        # real-time landing slots
        with tc.tile_wait_until(0.105):
            GP.dma_start(out=maskt_sb[:], in_=maskt[:])
            GP.dma_start(out=idbf_sb[:], in_=idbf[:])
        with tc.tile_wait_until(0.115):
            SYNC.dma_start(out=wv_sb[:], in_=wv[:])
        for b in range(4):
            with tc.tile_wait_until(0.130 + 0.010 * b):
                (GP, SYNC, GP, SYNC)[b].dma_start(
                    out=vdma_tiles[b][:],
                    in_=xvt[:, b * 4096 : (b + 1) * 4096],
                )
        with tc.tile_wait_until(0.210):
            GP.dma_start(out=consts_sb[:], in_=consts[:])
            for p in range(2):
                (SYNC, GP)[p].dma_start(
                    out=wo_sb[p][:], in_=wo[p * 128 : (p + 1) * 128, :]
                )

        # ---- Q then K projection, e-tile paced, full m0+m1 ----
        psA0_ctx = tc.tile_pool(name="psA0", bufs=1, space="PSUM")
        psA0 = psA0_ctx.__enter__()
        psA1_ctx = tc.tile_pool(name="psA1", bufs=1, space="PSUM", side="right")
        psA1 = psA1_ctx.__enter__()

        def proj_mm(ps, w_tile, m, e, xe, n):
            lhsT = w_tile[:, e * 256 + m * 128 : e * 256 + (m + 1) * 128]
            nc.tensor.matmul(
                ps[:, n * 512 : (n + 1) * 512],
                lhsT,
                xe[:, n * 512 : (n + 1) * 512],
                start=(e == 0),
                stop=(e == ET - 1),
            )

        def drain_chunk(eng, dst_t, ps, n):
            if eng is nc.scalar:
                eng.copy(dst_t[:, n * 512 : (n + 1) * 512],
                         ps[:, n * 512 : (n + 1) * 512])
            else:
                eng.tensor_copy(dst_t[:, n * 512 : (n + 1) * 512],
                                ps[:, n * 512 : (n + 1) * 512])

        # PSUM WAR is tracked at tile granularity, so a drain emitted
        # between two matmuls on the SAME psum tile serializes the PE.
        # Hide m0's drains under m1's e7 matmuls (different tile), and
        # m1's under the next phase's first m0 matmuls.
        for ti, (w_tile, dst) in enumerate(
            ((wq_sb, qt_sb), (wk_sb, kt_sb))
        ):
            ps0 = psA0.tile([128, S], f32, name=f"ps_p{ti}_0", tag="projA", bufs=1)
            ps1 = psA1.tile([128, S], f32, name=f"ps_p{ti}_1", tag="projB", bufs=1)
            for e in range(ET):
                xe = xe_tiles[(ti, e)]
                if e < ET - 1:
                    for m, ps in ((0, ps0), (1, ps1)):
                        for n in range(QT):
                            proj_mm(ps, w_tile, m, e, xe, n)
                else:
                    for n in range(QT):
                        proj_mm(ps0, w_tile, 0, e, xe, n)
                    # split drains across Vector and Scalar: both are
                    # idle pre-stream, and the first scores is gated on
                    # ALL m0 drains (PSUM WAR is tile-granular)
                    d_eng = (nc.vector, nc.scalar, nc.vector, nc.scalar)
                    for n in range(QT):
                        proj_mm(ps1, w_tile, 1, e, xe, n)
                        drain_chunk(d_eng[n], dst[0], ps0, n)
                    for n in range(QT):
                        drain_chunk(d_eng[3 - n], dst[1], ps1, n)

        # x e-tiles are dead once the projections are emitted; free the
        # pool so the stream pools below can overlay its SBUF.
        xep_ctx.__exit__(None, None, None)

        # hand the Q/K PSUM banks to the stream pools
        psA0_ctx.__exit__(None, None, None)
        psS_ctx = tc.tile_pool(name="psS", bufs=2, space="PSUM")
        psS = psS_ctx.__enter__()
        psA1_ctx.__exit__(None, None, None)
        psO_ctx = tc.tile_pool(name="psO", bufs=1, space="PSUM")
        psO = psO_ctx.__enter__()
        psV_ctx = tc.tile_pool(name="psV", bufs=2, space="PSUM")
        psV = psV_ctx.__enter__()
        psY = None  # opens once psV closes

        etp_ctx = tc.tile_pool(name="et", bufs=6)
        etp = etp_ctx.__enter__()
        obp_ctx = tc.tile_pool(name="ob", bufs=4)
        obp = obp_ctx.__enter__()
        rcp_ctx = tc.tile_pool(name="rcsb", bufs=3)
        rcp = rcp_ctx.__enter__()
        ysb_ctx = tc.tile_pool(name="ysb", bufs=3)
        ysbp = ysb_ctx.__enter__()

        nvp = 0  # V-projection k-tiles emitted

        def emit_vproj_tile():
            nonlocal nvp
            i = nvp
            psv = psV.tile([128, 256], f32, name=f"psv_{i}", tag="v")
            xvk = vdma_tiles[i // 4]
            k0 = (i % 4) * 128
            for e in range(ET):
                nc.tensor.matmul(
                    psv[:],
                    xvk[:, e * 512 + k0 : e * 512 + k0 + 128],
                    wv_sb[:, e * 256 : (e + 1) * 256],
                    start=(e == 0),
                    stop=(e == ET - 1),
                )
            nc.vector.tensor_copy(
                v_sb[i][:].rearrange("p (h c) -> p h c", c=128)[:, :, 0:64],
                psv[:].rearrange("p (h d) -> p h d", d=64),
            )
            nc.vector.tensor_copy(
                v_sb[i][:].rearrange("p (h c) -> p h c", c=128)[:, :, 64:65],
                idbf_sb[:, 128:132].rearrange("p (h c) -> p h c", c=1),
            )
            nvp += 1

        def emit_warm_filler(count):
            # dependency-free dummy matmuls: keep the PE clock up across
            # unavoidable serial waits (final normalize chain)
            wt2 = psS.tile([128, 1024], f32, name="warm2", tag="s")
            for _ in range(count):
                nc.tensor.matmul(
                    wt2[:, 0:256], idbf_sb[:, 0:128], maskt_sb[:, 0:256],
                    start=True, stop=True,
                )

        ysb_tiles = {}

        def emit_outproj_half(m, n, tail=False):
            psy = psY.tile([128, 512], f32, name=f"psy_{m}_{n}", tag="y")
            for p in range(2):
                nc.tensor.matmul(
                    psy[:],
                    outt_sb[p][:, m * 128 : (m + 1) * 128],
                    wo_sb[p][:, n * 512 : (n + 1) * 512],
                    start=(p == 0),
                    stop=(p == 1),
                )
            if n == 0:
                ysb_tiles[m] = ysbp.tile(
                    [128, 1024], bf16, name=f"y_sb_{m}", tag="ysb"
                )
            y_sb = ysb_tiles[m]
            if tail and n == 0:  # split tail casts across Act and Vector
                nc.scalar.copy(y_sb[:, n * 512 : (n + 1) * 512], psy[:])
            else:
                nc.vector.tensor_copy(y_sb[:, n * 512 : (n + 1) * 512], psy[:])
            if n == 1:
                eng = nc.gpsimd if m % 2 else nc.sync
                eng.dma_start(out=y[m * 128 : (m + 1) * 128, :], in_=y_sb[:])

        def emit_normalize(pr, jj, ob):
            # sums live on row 64 of ob for each head's 512-col half.
            ssb = rcp.tile([33, 512], f32, name=f"ssb_{pr}_{jj}", tag="ssb")
            for hh in range(2):
                nc.vector.tensor_copy(
                    ssb[32 * hh : 32 * hh + 1, :],
                    ob[64:65, 512 * hh : 512 * (hh + 1)],
                )
            rc32 = rcp.tile([33, 512], f32, name=f"rc32_{pr}_{jj}", tag="rc32")
            nc.vector.reciprocal_approx_fast(out=rc32[:], in_=ssb[:])
            rc = rcp.tile([33, 512], f32r, name=f"rc_{pr}_{jj}", tag="rc")
            nc.vector.tensor_copy(rc[:], rc32[:])
            bcs = []
            for hh in range(2):
                bc = psY.tile([128, 512], f32, name=f"ps_bc_{pr}_{jj}_{hh}", tag="y")
                nc.tensor.matmul(
                    bc[0:64, :],
                    consts_sb[32 * hh : 32 * hh + 1, 128:192],
                    rc[32 * hh : 32 * hh + 1, :],
                    start=True,
                    stop=True,
                    tile_position=(32 * hh, 0),
                )
                bcs.append(bc)
            for mo in range(4):
                for hh in range(2):
                    nc.vector.tensor_mul(
                        outt_sb[pr][64 * hh : 64 * hh + 64,
                                    jj * 512 + 128 * mo : jj * 512 + 128 * mo + 128],
                        ob[0:64, 512 * hh + 128 * mo : 512 * hh + 128 * mo + 128],
                        bcs[hh][0:64, 128 * mo : 128 * mo + 128],
                    )

        # ---- the flat attention stream ----
        # Alternate head-pairs so outproj block j unlocks after pass 2j+1.
        passes = [(0, 0), (1, 0), (0, 1), (1, 1), (0, 2), (1, 2), (0, 3), (1, 3)]
        tail_sums = rcp.tile([1, 1024], f32, name="tail_sums", tag="tsums")
        tail_rc32 = rcp.tile([1, 1024], f32, name="tail_rc32", tag="trc32")
        tail_rc = rcp.tile([1, 1024], f32r, name="tail_rc", tag="trc")
        steps = []
        for pr, j in passes:
            for i in range(4 * j + 4):
                steps.append((pr, j, i))

        # earliest step at which V-proj k-tile t may be emitted (xv batch
        # t//4 must have landed).  Compressed: psY (outproj PSUM) can only
        # open once psV closes, and the alternating pass order needs
        # outproj from ~step 18.
        vp_sched = {i: i + 2 for i in range(KT)}

        ps_outs = {}   # (pr, j) -> psO tile
        ob_tiles = {}  # (pr, j) -> SBUF drain tile
        pending = []   # emitted exps awaiting their attnV
        norm_q = []    # (ready_step, pr, j)
        outp_q = []    # (ready_step, m, n) output-projection halves
        pop_hold = 0   # extra pop delay right after a drain (psO WAR)

        def emit_attnv(pr, j, i, et, c0, n_i):
            nonlocal pop_hold
            if (pr, j) not in ps_outs:
                ps_outs[(pr, j)] = psO.tile(
                    [128, 1024], f32, name=f"ps_out_{pr}_{j}", tag="o"
                )
            ps_out = ps_outs[(pr, j)]
            for hh in range(2):
                nc.tensor.matmul(
                    ps_out[:, 512 * hh + c0 : 512 * (hh + 1)],
                    v_sb[i][:, (2 * pr + hh) * 128 : (2 * pr + hh + 1) * 128],
                    et[:, 512 * hh + c0 : 512 * (hh + 1)],
                    start=(i == 0),
                    stop=(i == n_i - 1),
                    skip_group_check=True,
                )
            if i == n_i - 1:
                # pass complete: drain the accumulator and queue the
                # (fully deferrable) normalize
                if (pr, j) == passes[-1]:
                    # tail: per-head 512-col chunks so the first bc can
                    # fire ~1.7us after the last attnV; out rows drain on
                    # Scalar (idle after the last exp)
                    for hh in range(2):
                        cs = slice(512 * hh, 512 * (hh + 1))
                        nc.vector.tensor_copy(tail_sums[:, cs], ps_out[64:65, cs])
                        nc.vector.reciprocal_approx_fast(
                            out=tail_rc32[:, cs], in_=tail_sums[:, cs]
                        )
                        nc.vector.tensor_copy(tail_rc[:, cs], tail_rc32[:, cs])
                    ob = obp.tile([64, 1024], f32, name="ob_tail", tag="obt")
                    nc.scalar.copy(ob[:], ps_out[0:64, :])
                else:
                    ob = obp.tile([65, 1024], f32, name=f"ob_{pr}_{j}", tag="ob")
                    nc.vector.tensor_copy(ob[:], ps_out[0:65, :])
                ob_tiles[(pr, j)] = ob
                pop_hold = 1

        for sidx, (pr, j, i) in enumerate(steps):
            n_i = 4 * j + 4
            # scores (+ causal mask straddle) and exp
            diag = i >= 4 * j
            r = i - 4 * j
            c0 = 128 * r if diag else 0
            qs = slice(j * 512, (j + 1) * 512)
            pss = psS.tile([128, 1024], f32, name=f"ps_s{pr}_{j}_{i}", tag="s")
            if diag:
                for hh in range(2):
                    nc.tensor.matmul(
                        pss[:, 512 * hh + c0 : 512 * hh + c0 + 128],
                        idbf_sb[:, 0:128],
                        maskt_sb[:, r * 256 : r * 256 + 128],
                        start=True,
                        stop=False,
                    )
            for hh in range(2):
                hp = slice(64 * hh, 64 * hh + 64)
                nc.tensor.matmul(
                    pss[:, 512 * hh + c0 : 512 * (hh + 1)],
                    kt_sb[pr][hp, i * 128 : (i + 1) * 128],
                    qt_sb[pr][hp, qs.start + c0 : qs.stop],
                    start=not diag,
                    stop=True,
                    skip_group_check=diag,
                )
            et = etp.tile([128, 1024], bf16, name=f"et{pr}_{j}_{i}", tag="et")
            nc.scalar.activation(et[:, c0:1024], pss[:, c0:1024], Exp, scale=0.125)
            pending.append((pr, j, i, et, c0, n_i))
            if nvp < KT and vp_sched[nvp] <= sidx:
                emit_vproj_tile()
            if len(pending) >= 3 + pop_hold:
                emit_attnv(*pending.pop(0))
                if len(pending) >= 4:  # catch up after a delayed start
                    emit_attnv(*pending.pop(0))
            elif pop_hold:
                pop_hold = 0
            if i == n_i - 1:
                norm_q.append((sidx + 3, pr, j))

            # PE filler after this step's main work
            if (
                norm_q
                and norm_q[0][0] <= sidx
                and psY is not None
                and tuple(norm_q[0][1:]) in ob_tiles
            ):
                _, npr, nj = norm_q.pop(0)
                emit_normalize(npr, nj, ob_tiles.pop((npr, nj)))
                if npr == 1:
                    for mo in range(4):
                        for n in range(2):
                            outp_q.append((sidx + 2 + mo, 4 * nj + mo, n))
            if outp_q and outp_q[0][0] <= sidx and psY is not None:
                _, m, n = outp_q.pop(0)
                emit_outproj_half(m, n)
                # one more half if backlogged
                if outp_q and outp_q[0][0] + 2 <= sidx:
                    _, m, n = outp_q.pop(0)
                    emit_outproj_half(m, n)
            if nvp == KT and psY is None:
                psV_ctx.__exit__(None, None, None)
                psY_ctx = tc.tile_pool(name="psY", bufs=2, space="PSUM")
                psY = psY_ctx.__enter__()

        # tail: flush remaining attnVs, then a pipelined normalize +
        # outproj chain, with warm fillers covering the serial prefix so
        # the PE clock gate never drops.
        while pending:
            emit_attnv(*pending.pop(0))
        emit_warm_filler(28)
        while outp_q:
            _, m, n = outp_q.pop(0)
            emit_outproj_half(m, n)
        while norm_q:
            _, npr, nj = norm_q.pop(0)
            if (npr, nj) == passes[-1]:
                # the recip chain already ran (chunked) off the last
                # attnV; bc from partition 0; each 128-col chunk chased
                # by its outproj
                ob = ob_tiles.pop((npr, nj))
                bcs = []
                for hh in range(2):
                    bc = psY.tile([128, 512], f32, name=f"t_bc_{hh}", tag="y")
                    nc.tensor.matmul(
                        bc[0:64, :],
                        consts_sb[0:1, 128:192],
                        tail_rc[0:1, 512 * hh : 512 * (hh + 1)],
                        start=True,
                        stop=True,
                    )
                    bcs.append(bc)
                for mo in range(4):
                    for hh in range(2):
                        nc.vector.tensor_mul(
                            outt_sb[npr][64 * hh : 64 * hh + 64,
                                         nj * 512 + 128 * mo : nj * 512 + 128 * mo + 128],
                            ob[0:64, 512 * hh + 128 * mo : 512 * hh + 128 * mo + 128],
                            bcs[hh][0:64, 128 * mo : 128 * mo + 128],
                        )
                    emit_outproj_half(4 * nj + mo, 0, tail=True)
                    emit_outproj_half(4 * nj + mo, 1, tail=True)
            else:
                emit_normalize(npr, nj, ob_tiles.pop((npr, nj)))
                if npr == 1:
                    for mo in range(4):
                        emit_outproj_half(4 * nj + mo, 0)
                        emit_outproj_half(4 * nj + mo, 1)

        for ctx in (ysb_ctx, rcp_ctx, obp_ctx, etp_ctx):
            ctx.__exit__(None, None, None)
        psY_ctx.__exit__(None, None, None)
        psO_ctx.__exit__(None, None, None)
        psS_ctx.__exit__(None, None, None)
        xep_ctx.__exit__(None, None, None)
        xvkp_ctx.__exit__(None, None, None)

    nc.compile()
    return nc


def _get_program():
    if "nc" not in _PROG_CACHE:
        _PROG_CACHE["nc"] = _build_program()
    return _PROG_CACHE["nc"]


def _host_prep(query, key, value, mask, w_q, w_k, w_v, w_o):
    import ml_dtypes

    bf = ml_dtypes.bfloat16
    query = np.asarray(query, dtype=np.float32)
    key = np.asarray(key, dtype=np.float32)
    value = np.asarray(value, dtype=np.float32)
    w_q = np.asarray(w_q, dtype=np.float32)
    w_k = np.asarray(w_k, dtype=np.float32)
    w_v = np.asarray(w_v, dtype=np.float32)
    w_o = np.asarray(w_o, dtype=np.float32)
    m = np.asarray(mask).reshape(S, S).astype(bool)

    # The kernel's block-skip structure assumes the standard causal mask.
    expected = np.triu(np.ones((S, S), dtype=bool), k=1)
    if not np.array_equal(m, expected):
        raise NotImplementedError("kernel specialized for causal (triu, k=1) mask")

    # 4 canonical diagonal-straddle mask tiles: pattern r covers k-tile
    # 4j+r vs q-tile j; masked where (128r + row) > col.
    maskt = np.zeros((128, 1024), dtype=np.float32)
    rows = np.arange(128)[:, None]
    cols = np.arange(128)[None, :]
    straddle = np.where(rows > cols, np.float32(-1e9), np.float32(0.0))
    for r in range(4):
        maskt[:, r * 256 : r * 256 + 128] = straddle
        maskt[:, r * 256 + 128 : r * 256 + 256] = straddle
    maskt = maskt.astype(bf)
    idbf = np.zeros((128, 132), dtype=bf)
    idbf[:, 0:128] = np.eye(128, dtype=bf)
    idbf[:, 128:132] = bf(1.0)

    consts = np.zeros((128, 193), dtype=np.float32)
    consts[:, 0:128] = np.eye(128, dtype=np.float32)
    consts[:, 128:193] = 1.0

    def tile_w(w_rows):  # [256, D_MODEL] -> [128, ET*256] (p (t d))
        return np.ascontiguousarray(
            w_rows.T.reshape(ET, 128, 256).transpose(1, 0, 2).reshape(128, ET * 256)
        ).astype(bf)

    xt = {}
    for b in range(B):
        xt[("q", b)] = np.ascontiguousarray(query[b].T.astype(bf))
        xt[("k", b)] = np.ascontiguousarray(key[b].T.astype(bf))
        xv = value[b].T.astype(bf)  # [D_MODEL, S]
        # batch-major pretile: [128, 4*ET*512], batch b4 block contiguous
        xvt = (
            xv.reshape(ET, 128, 4, 512)
            .transpose(1, 2, 0, 3)  # [128, 4, ET, 512]
            .reshape(128, 4 * ET * 512)
        )
        xt[("v", b)] = np.ascontiguousarray(xvt)

    in_maps = []
    for c in range(N_CORES):
        b = c // 4
        hb = (c % 4) * HPC
        rs = slice(hb * D_K, (hb + HPC) * D_K)
        in_maps.append(
            {
                "xq": xt[("q", b)],
                "xk": xt[("k", b)],
                "xvt": xt[("v", b)],
                "wq": tile_w(w_q[rs, :]),
                "wk": tile_w(w_k[rs, :]),
                "wv": tile_w(w_v[rs, :]),
                "wo": np.ascontiguousarray(w_o[:, rs].T.astype(bf)),
                "maskt": maskt,
                "idbf": idbf,
                "consts": consts,
            }
        )
    return in_maps


def kernel(query, key, value, mask, w_q, w_k, w_v, w_o):
    from concourse.bass_utils import run_bass_kernel_spmd

    in_maps = _host_prep(query, key, value, mask, w_q, w_k, w_v, w_o)
    nc = _get_program()
    res = run_bass_kernel_spmd(nc, in_maps, list(range(N_CORES)))
    out = np.zeros((B, S, D_MODEL), dtype=np.float32)
    for c in range(N_CORES):
        out[c // 4] += np.asarray(res.results[c]["y"], dtype=np.float32)
    return out


# revision 53
# speedup vs baseline: 1.0818x; 1.0818x over previous
"""Multi-head causal attention (B=2, S=2048, D=1024, H=16) on 8 TRN2 NeuronCores.

Sharding: batch*head parallel. Core c handles batch b = c//4 and the 4
heads h in [4*(c%4), 4*(c%4)+4). Each core computes its heads' Q/K/V
projections (column-parallel), causal softmax attention, and its partial
row-parallel output projection; the host sums the 4 partial outputs per
batch (the AllReduce of row-parallel tensor parallelism).

On-device layout: everything is kept "transposed" (feature-major) so
every matmul contracts along the partition dimension:
  scoresT[k,q] = K Q^T      (per head, 128-row k-tiles x 512-col q-tiles)
  P^T = exp(scoresT/8 + mask/8)   (additive -1e9 causal mask)
  outT[d,q]   = sum_k V[k,d] P^T[k,q]   (PSUM-accumulated over k-tiles)
  sums[q]     = sum_k P^T[k,q]          (ones-vector matmul, col-packed)
  y[q,e]     += sum_hd outT_norm[hd,q] * w_oT[hd,e]
Softmax skips the max-subtraction: scores ~ N(0,1) so exp never
overflows, and exp(-1e9/8) underflows to exactly 0 like the
reference's masked_fill(-1e9).

Schedule (v2, rebuilt from the measured baseline trace):
- All DRAM operands are HOST-pretiled so every input DMA moves >=4KB
  contiguous lines (engine descriptor-gen cost ~0.65us instead of
  2-7us for strided patterns).
- DMA priority order: wq, xq, wk, xk (the 9MB that gates the attention
  stream), then maskt/idbf/wv/xvt/consts/wo, round-robin over the
  sync/gpsimd/scalar queues. The Scalar queue carries only early
  ungated loads so nothing blocks the exp stream.
- Front: warm-up matmuls (PE clock-gate ramp), then Q projection
  (m0+m1 per e-tile, DMA-paced, 8 PSUM banks), drains chunked per
  512 cols across Vector/Scalar/GpSimd, then K projection the same
  way reusing the banks. First scores fire ~0.6us after the first
  K-m0 drain chunk.
- Stream: flat software-pipelined attention as in the baseline, with
  passes ALTERNATING head-pairs ((0,0),(1,0),(0,1),(1,1),...) so the
  row-parallel output projection (needs both pairs per q-block)
  spreads through the stream instead of piling at the end.
- ob accumulator drains ride GpSimd (idle during the stream); psy
  casts ride Vector; y DMAs GpSimd (tail ones alternate with Scalar,
  which is idle after the last exp).
- Tail: the last pass's normalize is interleaved per-128-col chunk
  with its output-projection matmuls, casts and DMAs so the post-exp
  tail is a pipeline, not a serial chain.
"""

import numpy as np

D_MODEL = 1024
N_HEADS = 16
D_K = 64
B, S = 2, 2048
N_CORES = 8
HPC = 4            # heads per core
KT = S // 128      # 16 k-tiles
QT = S // 512      # 4 q-tiles
ET = D_MODEL // 128  # 8 e-tiles (contraction tiles for projections)

_PROG_CACHE = {}


def _build_program():
    import concourse.bacc as bacc_mod
    import concourse.mybir as mybir
    import concourse.tile as tile

    f32 = mybir.dt.float32
    f32r = mybir.dt.float32r
    bf16 = mybir.dt.bfloat16
    Exp = mybir.ActivationFunctionType.Exp

    nc = bacc_mod.Bacc(
        "TRN2", target_bir_lowering=False, debug=False, num_devices=N_CORES
    )

    xq = nc.dram_tensor("xq", [D_MODEL, S], bf16, kind="ExternalInput").ap()
    xk = nc.dram_tensor("xk", [D_MODEL, S], bf16, kind="ExternalInput").ap()
    xvt = nc.dram_tensor("xvt", [128, 4 * ET * 512], bf16, kind="ExternalInput").ap()
    wq = nc.dram_tensor("wq", [128, ET * 256], bf16, kind="ExternalInput").ap()
    wk = nc.dram_tensor("wk", [128, ET * 256], bf16, kind="ExternalInput").ap()
    wv = nc.dram_tensor("wv", [128, ET * 256], bf16, kind="ExternalInput").ap()
    wo = nc.dram_tensor("wo", [256, D_MODEL], bf16, kind="ExternalInput").ap()
    maskt = nc.dram_tensor("maskt", [128, 1024], bf16, kind="ExternalInput").ap()
    idbf = nc.dram_tensor("idbf", [128, 132], bf16, kind="ExternalInput").ap()
    consts = nc.dram_tensor("consts", [128, 193], f32r, kind="ExternalInput").ap()
    y = nc.dram_tensor("y", [S, D_MODEL], bf16, kind="ExternalOutput").ap()

    with (
        tile.TileContext(nc) as tc,
        nc.allow_low_precision("bf16 attention"),
        tc.tile_pool(name="persist", bufs=1) as pp,
    ):
        # ---- persistent SBUF tiles ----
        def persist(shape, dtype, name):
            return pp.tile(shape, dtype, name=name, tag=name)

        wq_sb = persist([128, ET * 256], bf16, "wq_sb")
        wk_sb = persist([128, ET * 256], bf16, "wk_sb")
        wv_sb = persist([128, ET * 256], bf16, "wv_sb")
        wo_sb = [persist([128, D_MODEL], bf16, f"wo_sb{p}") for p in range(2)]
        maskt_sb = persist([128, 1024], bf16, "maskt_sb")
        idbf_sb = persist([128, 132], bf16, "idbf_sb")
        consts_sb = persist([128, 193], f32r, "consts_sb")
        qt_sb = [persist([128, S], bf16, f"qt_sb{p}") for p in range(2)]
        kt_sb = [persist([128, S], bf16, f"kt_sb{p}") for p in range(2)]
        # 128 cols per head (64 V + 1 ones + 63 zeros): the attnV
        # stationary is then exactly 128 weight-rows (FWL trigger).
        v_sb = [persist([128, 512], bf16, f"v_sb{i}") for i in range(KT)]
        outt_sb = [persist([128, S], bf16, f"outt_sb{p}") for p in range(2)]

        # ---- PE warm-up ----
        # The PE HAM clock gate drops to K=4 half-clock after any multi-us
        # PE idle and needs ~3.4us of gapless activity to return to K=8.
        # Dummy matmuls on a memset tile cover the ramp until the first
        # projection e-tile lands (~9.5us: preamble ~7us + wq + xq chunk).
        warm_src = persist([128, 640], bf16, "warm_src")
        nc.vector.memset(warm_src[:], 0.0)
        for i in range(KT):
            nc.vector.memset(v_sb[i][:], 0.0)
        with tc.tile_pool(name="psW", bufs=1, space="PSUM") as psW:
            wt = psW.tile([128, 512], f32, name="warm", tag="warm")
            for w in range(22):
                nc.tensor.matmul(
                    wt[:], warm_src[:, 0:128], warm_src[:, 128:640],
                    start=True, stop=True,
                )

        # ---- DMA issue: everything up front, in priority order ----
        # Three queues (sync HWDGE, gpsimd SWDGE, scalar HWDGE) round-
        # robin.  The stream gate is wq+xq+wk+xk (9MB ~ 25us at HBM BW);
        # everything else follows.
        # all 16 x e-tiles live at once: no WAR gating on any input DMA
        # issue, so the three queues stream the 9MB gate continuously
        xvkp_ctx = tc.tile_pool(name="xvk", bufs=4)
        xvkp = xvkp_ctx.__enter__()
        xep_ctx = tc.tile_pool(name="xe", bufs=16, side="right")
        xep = xep_ctx.__enter__()

        SYNC, GP, SC = nc.sync, nc.gpsimd, nc.scalar
        vdma_tiles = [
            xvkp.tile([128, ET * 512], bf16, name=f"xvk_{b}", tag="xvk")
            for b in range(4)
        ]
        xe_tiles = {}
        for ti in range(2):
            for e in range(ET):
                xe_tiles[(ti, e)] = xep.tile(
                    [128, S], bf16, name=f"xe_{ti}_{e}", tag="xe"
                )

        # The tile scheduler reorders same-engine DMA issues; wait-hints
        # (scheduler-time lower bounds) keep everything that is not the
        # stream gate (wq+xq+wk+xk) behind it in HBM-bandwidth order.
        # wq + the first e-tile ride Sync (HW-DGE) back-to-back so the
        # first projection matmul can start ~11us.
        # wq in halves + first xq e-tile in 4 chunks, spread across the
        # queues so the first matmul starts ~10us (one queue alone is
        # only ~110GB/s and a monolithic wq would gate Q-e0 ~5us)
        SYNC.dma_start(out=wq_sb[:, 0:1024], in_=wq[:, 0:1024])
        GP.dma_start(out=wq_sb[:, 1024:2048], in_=wq[:, 1024:2048])
        for c in range(4):
            (SC, SYNC, GP, SC)[c].dma_start(
                out=xe_tiles[(0, 0)][:, c * 512 : (c + 1) * 512],
                in_=xq[0:128, c * 512 : (c + 1) * 512],
            )
        xq_engs = (SC, SYNC, GP, SC, SYNC, GP, SC)
        for e in range(1, ET):
            xq_engs[e - 1].dma_start(
                out=xe_tiles[(0, e)][:], in_=xq[e * 128 : (e + 1) * 128, :]
            )
        SYNC.dma_start(out=wk_sb[:], in_=wk[:])
        xk_engs = (SC, SYNC, SC, GP, SYNC, SC, SYNC, GP)
        for e in range(ET):
            xk_engs[e].dma_start(
                out=xe_tiles[(1, e)][:], in_=xk[e * 128 : (e + 1) * 128, :]
            )
        # wait-hints (scheduler-time lower bounds) keep everything that
        # is not the stream gate behind it in HBM-bandwidth order
        with tc.tile_wait_until(0.026):
            GP.dma_start(out=maskt_sb[:], in_=maskt[:])
            GP.dma_start(out=idbf_sb[:], in_=idbf[:])
        with tc.tile_wait_until(0.030):
            SYNC.dma_start(out=wv_sb[:], in_=wv[:])
        for b in range(4):
            with tc.tile_wait_until(0.033 + 0.004 * b):
                (GP, SYNC, GP, SYNC)[b].dma_start(
                    out=vdma_tiles[b][:],
                    in_=xvt[:, b * 4096 : (b + 1) * 4096],
                )
        with tc.tile_wait_until(0.050):
            GP.dma_start(out=consts_sb[:], in_=consts[:])
            for p in range(2):
                (SYNC, GP)[p].dma_start(
                    out=wo_sb[p][:], in_=wo[p * 128 : (p + 1) * 128, :]
                )

        # ---- Q then K projection, e-tile paced, full m0+m1 ----
        psA0_ctx = tc.tile_pool(name="psA0", bufs=1, space="PSUM")
        psA0 = psA0_ctx.__enter__()
        psA1_ctx = tc.tile_pool(name="psA1", bufs=1, space="PSUM", side="right")
        psA1 = psA1_ctx.__enter__()

        def proj_mm(ps, w_tile, m, e, xe, n):
            lhsT = w_tile[:, e * 256 + m * 128 : e * 256 + (m + 1) * 128]
            nc.tensor.matmul(
                ps[:, n * 512 : (n + 1) * 512],
                lhsT,
                xe[:, n * 512 : (n + 1) * 512],
                start=(e == 0),
                stop=(e == ET - 1),
            )

        def drain_chunk(eng, dst_t, ps, n):
            if eng is nc.scalar:
                eng.copy(dst_t[:, n * 512 : (n + 1) * 512],
                         ps[:, n * 512 : (n + 1) * 512])
            else:
                eng.tensor_copy(dst_t[:, n * 512 : (n + 1) * 512],
                                ps[:, n * 512 : (n + 1) * 512])

        # PSUM WAR is tracked at tile granularity, so a drain emitted
        # between two matmuls on the SAME psum tile serializes the PE.
        # Hide m0's drains under m1's e7 matmuls (different tile), and
        # m1's under the next phase's first m0 matmuls.
        for ti, (w_tile, dst) in enumerate(
            ((wq_sb, qt_sb), (wk_sb, kt_sb))
        ):
            ps0 = psA0.tile([128, S], f32, name=f"ps_p{ti}_0", tag="projA", bufs=1)
            ps1 = psA1.tile([128, S], f32, name=f"ps_p{ti}_1", tag="projB", bufs=1)
            for e in range(ET):
                xe = xe_tiles[(ti, e)]
                if e < ET - 1:
                    for m, ps in ((0, ps0), (1, ps1)):
                        for n in range(QT):
                            proj_mm(ps, w_tile, m, e, xe, n)
                else:
                    for n in range(QT):
                        proj_mm(ps0, w_tile, 0, e, xe, n)
                    # K drains stay off Scalar so nothing queues ahead
                    # of the first exp
                    d_eng = (
                        (nc.vector, nc.scalar, nc.vector, nc.scalar)
                        if ti == 0
                        else (nc.vector, nc.vector, nc.vector, nc.vector)
                    )
                    for n in range(QT):
                        proj_mm(ps1, w_tile, 1, e, xe, n)
                        drain_chunk(d_eng[n], dst[0], ps0, n)
                    for n in range(QT):
                        drain_chunk(d_eng[n], dst[1], ps1, n)

        # x e-tiles are dead once the projections are emitted; free the
        # pool so the stream pools below can overlay its SBUF.
        xep_ctx.__exit__(None, None, None)

        # hand the Q/K PSUM banks to the stream pools
        psA0_ctx.__exit__(None, None, None)
        psS_ctx = tc.tile_pool(name="psS", bufs=2, space="PSUM")
        psS = psS_ctx.__enter__()
        psA1_ctx.__exit__(None, None, None)
        psO_ctx = tc.tile_pool(name="psO", bufs=1, space="PSUM")
        psO = psO_ctx.__enter__()
        psV_ctx = tc.tile_pool(name="psV", bufs=2, space="PSUM")
        psV = psV_ctx.__enter__()
        psY = None  # opens once psV closes

        etp_ctx = tc.tile_pool(name="et", bufs=6)
        etp = etp_ctx.__enter__()
        obp_ctx = tc.tile_pool(name="ob", bufs=4)
        obp = obp_ctx.__enter__()
        rcp_ctx = tc.tile_pool(name="rcsb", bufs=3)
        rcp = rcp_ctx.__enter__()
        ysb_ctx = tc.tile_pool(name="ysb", bufs=3)
        ysbp = ysb_ctx.__enter__()

        nvp = 0  # V-projection k-tiles emitted

        def emit_vproj_tile():
            nonlocal nvp
            i = nvp
            psv = psV.tile([128, 256], f32, name=f"psv_{i}", tag="v")
            xvk = vdma_tiles[i // 4]
            k0 = (i % 4) * 128
            for e in range(ET):
                nc.tensor.matmul(
                    psv[:],
                    xvk[:, e * 512 + k0 : e * 512 + k0 + 128],
                    wv_sb[:, e * 256 : (e + 1) * 256],
                    start=(e == 0),
                    stop=(e == ET - 1),
                )
            nc.vector.tensor_copy(
                v_sb[i][:].rearrange("p (h c) -> p h c", c=128)[:, :, 0:64],
                psv[:].rearrange("p (h d) -> p h d", d=64),
            )
            nc.vector.tensor_copy(
                v_sb[i][:].rearrange("p (h c) -> p h c", c=128)[:, :, 64:65],
                idbf_sb[:, 128:132].rearrange("p (h c) -> p h c", c=1),
            )
            nvp += 1

        def emit_warm_filler(count):
            # dependency-free dummy matmuls: keep the PE clock up across
            # unavoidable serial waits (final normalize chain)
            wt2 = psS.tile([128, 1024], f32, name="warm2", tag="s")
            for _ in range(count):
                nc.tensor.matmul(
                    wt2[:, 0:256], idbf_sb[:, 0:128], maskt_sb[:, 0:256],
                    start=True, stop=True,
                )

        ysb_tiles = {}

        def emit_outproj_half(m, n, tail=False):
            psy = psY.tile([128, 512], f32, name=f"psy_{m}_{n}", tag="y")
            for p in range(2):
                nc.tensor.matmul(
                    psy[:],
                    outt_sb[p][:, m * 128 : (m + 1) * 128],
                    wo_sb[p][:, n * 512 : (n + 1) * 512],
                    start=(p == 0),
                    stop=(p == 1),
                )
            if n == 0:
                ysb_tiles[m] = ysbp.tile(
                    [128, 1024], bf16, name=f"y_sb_{m}", tag="ysb"
                )
            y_sb = ysb_tiles[m]
            if tail and n == 0:  # split tail casts across Act and Vector
                nc.scalar.copy(y_sb[:, n * 512 : (n + 1) * 512], psy[:])
            else:
                nc.vector.tensor_copy(y_sb[:, n * 512 : (n + 1) * 512], psy[:])
            if n == 1:
                eng = nc.gpsimd if m % 2 else nc.sync
                eng.dma_start(out=y[m * 128 : (m + 1) * 128, :], in_=y_sb[:])

        def emit_normalize(pr, jj, ob):
            # sums live on row 64 of ob for each head's 512-col half.
            ssb = rcp.tile([33, 512], f32, name=f"ssb_{pr}_{jj}", tag="ssb")
            for hh in range(2):
                nc.vector.tensor_copy(
                    ssb[32 * hh : 32 * hh + 1, :],
                    ob[64:65, 512 * hh : 512 * (hh + 1)],
                )
            rc32 = rcp.tile([33, 512], f32, name=f"rc32_{pr}_{jj}", tag="rc32")
            nc.vector.reciprocal_approx_fast(out=rc32[:], in_=ssb[:])
            rc = rcp.tile([33, 512], f32r, name=f"rc_{pr}_{jj}", tag="rc")
            nc.vector.tensor_copy(rc[:], rc32[:])
            bcs = []
            for hh in range(2):
                bc = psY.tile([128, 512], f32, name=f"ps_bc_{pr}_{jj}_{hh}", tag="y")
                nc.tensor.matmul(
                    bc[0:64, :],
                    consts_sb[32 * hh : 32 * hh + 1, 128:192],
                    rc[32 * hh : 32 * hh + 1, :],
                    start=True,
                    stop=True,
                    tile_position=(32 * hh, 0),
                )
                bcs.append(bc)
            for mo in range(4):
                for hh in range(2):
                    nc.vector.tensor_mul(
                        outt_sb[pr][64 * hh : 64 * hh + 64,
                                    jj * 512 + 128 * mo : jj * 512 + 128 * mo + 128],
                        ob[0:64, 512 * hh + 128 * mo : 512 * hh + 128 * mo + 128],
                        bcs[hh][0:64, 128 * mo : 128 * mo + 128],
                    )

        # ---- the flat attention stream ----
        # Alternate head-pairs so outproj block j unlocks after pass 2j+1.
        passes = [(0, 0), (1, 0), (0, 1), (1, 1), (0, 2), (1, 2), (0, 3), (1, 3)]
        tail_sums = rcp.tile([1, 1024], f32, name="tail_sums", tag="tsums")
        tail_rc32 = rcp.tile([1, 1024], f32, name="tail_rc32", tag="trc32")
        tail_rc = rcp.tile([1, 1024], f32r, name="tail_rc", tag="trc")
        steps = []
        for pr, j in passes:
            for i in range(4 * j + 4):
                steps.append((pr, j, i))

        # earliest step at which V-proj k-tile t may be emitted (xv batch
        # t//4 must have landed).  Compressed: psY (outproj PSUM) can only
        # open once psV closes, and the alternating pass order needs
        # outproj from ~step 18.
        vp_sched = {i: i + 2 for i in range(KT)}

        ps_outs = {}   # (pr, j) -> psO tile
        ob_tiles = {}  # (pr, j) -> SBUF drain tile
        pending = []   # emitted exps awaiting their attnV
        norm_q = []    # (ready_step, pr, j)
        outp_q = []    # (ready_step, m, n) output-projection halves
        pop_hold = 0   # extra pop delay right after a drain (psO WAR)

        def emit_attnv(pr, j, i, et, c0, n_i):
            nonlocal pop_hold
            if (pr, j) not in ps_outs:
                ps_outs[(pr, j)] = psO.tile(
                    [128, 1024], f32, name=f"ps_out_{pr}_{j}", tag="o"
                )
            ps_out = ps_outs[(pr, j)]
            for hh in range(2):
                nc.tensor.matmul(
                    ps_out[:, 512 * hh + c0 : 512 * (hh + 1)],
                    v_sb[i][:, (2 * pr + hh) * 128 : (2 * pr + hh + 1) * 128],
                    et[:, 512 * hh + c0 : 512 * (hh + 1)],
                    start=(i == 0),
                    stop=(i == n_i - 1),
                    skip_group_check=True,
                )
            if i == n_i - 1:
                # pass complete: drain the accumulator and queue the
                # (fully deferrable) normalize
                if (pr, j) == passes[-1]:
                    # tail: per-head 512-col chunks so the first bc can
                    # fire ~1.7us after the last attnV; out rows drain on
                    # Scalar (idle after the last exp)
                    for hh in range(2):
                        cs = slice(512 * hh, 512 * (hh + 1))
                        nc.vector.tensor_copy(tail_sums[:, cs], ps_out[64:65, cs])
                        nc.vector.reciprocal_approx_fast(
                            out=tail_rc32[:, cs], in_=tail_sums[:, cs]
                        )
                        nc.vector.tensor_copy(tail_rc[:, cs], tail_rc32[:, cs])
                    ob = obp.tile([64, 1024], f32, name="ob_tail", tag="obt")
                    nc.scalar.copy(ob[:], ps_out[0:64, :])
                else:
                    ob = obp.tile([65, 1024], f32, name=f"ob_{pr}_{j}", tag="ob")
                    nc.vector.tensor_copy(ob[:], ps_out[0:65, :])
                ob_tiles[(pr, j)] = ob
                pop_hold = 1

        for sidx, (pr, j, i) in enumerate(steps):
            n_i = 4 * j + 4
            # scores (+ causal mask straddle) and exp
            diag = i >= 4 * j
            r = i - 4 * j
            c0 = 128 * r if diag else 0
            qs = slice(j * 512, (j + 1) * 512)
            pss = psS.tile([128, 1024], f32, name=f"ps_s{pr}_{j}_{i}", tag="s")
            if diag:
                for hh in range(2):
                    nc.tensor.matmul(
                        pss[:, 512 * hh + c0 : 512 * hh + c0 + 128],
                        idbf_sb[:, 0:128],
                        maskt_sb[:, r * 256 : r * 256 + 128],
                        start=True,
                        stop=False,
                    )
            for hh in range(2):
                hp = slice(64 * hh, 64 * hh + 64)
                nc.tensor.matmul(
                    pss[:, 512 * hh + c0 : 512 * (hh + 1)],
                    kt_sb[pr][hp, i * 128 : (i + 1) * 128],
                    qt_sb[pr][hp, qs.start + c0 : qs.stop],
                    start=not diag,
                    stop=True,
                    skip_group_check=diag,
                )
            et = etp.tile([128, 1024], bf16, name=f"et{pr}_{j}_{i}", tag="et")
            nc.scalar.activation(et[:, c0:1024], pss[:, c0:1024], Exp, scale=0.125)
            pending.append((pr, j, i, et, c0, n_i))
            if nvp < KT and vp_sched[nvp] <= sidx:
                emit_vproj_tile()
            if len(pending) >= 3 + pop_hold:
                emit_attnv(*pending.pop(0))
                if len(pending) >= 4:  # catch up after a delayed start
                    emit_attnv(*pending.pop(0))
            elif pop_hold:
                pop_hold = 0
            if i == n_i - 1:
                norm_q.append((sidx + 3, pr, j))

            # PE filler after this step's main work
            if (
                norm_q
                and norm_q[0][0] <= sidx
                and psY is not None
                and tuple(norm_q[0][1:]) in ob_tiles
            ):
                _, npr, nj = norm_q.pop(0)
                emit_normalize(npr, nj, ob_tiles.pop((npr, nj)))
                if npr == 1:
                    for mo in range(4):
                        for n in range(2):
                            outp_q.append((sidx + 2 + mo, 4 * nj + mo, n))
            if outp_q and outp_q[0][0] <= sidx and psY is not None:
                _, m, n = outp_q.pop(0)
                emit_outproj_half(m, n)
                # one more half if backlogged
                if outp_q and outp_q[0][0] + 2 <= sidx:
                    _, m, n = outp_q.pop(0)
                    emit_outproj_half(m, n)
            if nvp == KT and psY is None:
                psV_ctx.__exit__(None, None, None)
                psY_ctx = tc.tile_pool(name="psY", bufs=2, space="PSUM")
                psY = psY_ctx.__enter__()

        # tail: flush remaining attnVs, then a pipelined normalize +
        # outproj chain, with warm fillers covering the serial prefix so
        # the PE clock gate never drops.
        while pending:
            emit_attnv(*pending.pop(0))
        emit_warm_filler(28)
        while outp_q:
            _, m, n = outp_q.pop(0)
            emit_outproj_half(m, n)
        while norm_q:
            _, npr, nj = norm_q.pop(0)
            if (npr, nj) == passes[-1]:
                # the recip chain already ran (chunked) off the last
                # attnV; bc from partition 0; each 128-col chunk chased
                # by its outproj
                ob = ob_tiles.pop((npr, nj))
                bcs = []
                for hh in range(2):
                    bc = psY.tile([128, 512], f32, name=f"t_bc_{hh}", tag="y")
                    nc.tensor.matmul(
                        bc[0:64, :],
                        consts_sb[0:1, 128:192],
                        tail_rc[0:1, 512 * hh : 512 * (hh + 1)],
                        start=True,
                        stop=True,
                    )
                    bcs.append(bc)
                for mo in range(4):
                    for hh in range(2):
                        nc.vector.tensor_mul(
                            outt_sb[npr][64 * hh : 64 * hh + 64,
                                         nj * 512 + 128 * mo : nj * 512 + 128 * mo + 128],
                            ob[0:64, 512 * hh + 128 * mo : 512 * hh + 128 * mo + 128],
                            bcs[hh][0:64, 128 * mo : 128 * mo + 128],
                        )
                    emit_outproj_half(4 * nj + mo, 0, tail=True)
                    emit_outproj_half(4 * nj + mo, 1, tail=True)
            else:
                emit_normalize(npr, nj, ob_tiles.pop((npr, nj)))
                if npr == 1:
                    for mo in range(4):
                        emit_outproj_half(4 * nj + mo, 0)
                        emit_outproj_half(4 * nj + mo, 1)

        for ctx in (ysb_ctx, rcp_ctx, obp_ctx, etp_ctx):
            ctx.__exit__(None, None, None)
        psY_ctx.__exit__(None, None, None)
        psO_ctx.__exit__(None, None, None)
        psS_ctx.__exit__(None, None, None)
        xep_ctx.__exit__(None, None, None)
        xvkp_ctx.__exit__(None, None, None)

    nc.compile()
    return nc


def _get_program():
    if "nc" not in _PROG_CACHE:
        _PROG_CACHE["nc"] = _build_program()
    return _PROG_CACHE["nc"]


def _host_prep(query, key, value, mask, w_q, w_k, w_v, w_o):
    import ml_dtypes

    bf = ml_dtypes.bfloat16
    query = np.asarray(query, dtype=np.float32)
    key = np.asarray(key, dtype=np.float32)
    value = np.asarray(value, dtype=np.float32)
    w_q = np.asarray(w_q, dtype=np.float32)
    w_k = np.asarray(w_k, dtype=np.float32)
    w_v = np.asarray(w_v, dtype=np.float32)
    w_o = np.asarray(w_o, dtype=np.float32)
    m = np.asarray(mask).reshape(S, S).astype(bool)

    # The kernel's block-skip structure assumes the standard causal mask.
    expected = np.triu(np.ones((S, S), dtype=bool), k=1)
    if not np.array_equal(m, expected):
        raise NotImplementedError("kernel specialized for causal (triu, k=1) mask")

    # 4 canonical diagonal-straddle mask tiles: pattern r covers k-tile
    # 4j+r vs q-tile j; masked where (128r + row) > col.
    maskt = np.zeros((128, 1024), dtype=np.float32)
    rows = np.arange(128)[:, None]
    cols = np.arange(128)[None, :]
    straddle = np.where(rows > cols, np.float32(-1e9), np.float32(0.0))
    for r in range(4):
        maskt[:, r * 256 : r * 256 + 128] = straddle
        maskt[:, r * 256 + 128 : r * 256 + 256] = straddle
    maskt = maskt.astype(bf)
    idbf = np.zeros((128, 132), dtype=bf)
    idbf[:, 0:128] = np.eye(128, dtype=bf)
    idbf[:, 128:132] = bf(1.0)

    consts = np.zeros((128, 193), dtype=np.float32)
    consts[:, 0:128] = np.eye(128, dtype=np.float32)
    consts[:, 128:193] = 1.0

    def tile_w(w_rows):  # [256, D_MODEL] -> [128, ET*256] (p (t d))
        return np.ascontiguousarray(
            w_rows.T.reshape(ET, 128, 256).transpose(1, 0, 2).reshape(128, ET * 256)
        ).astype(bf)

    xt = {}
    for b in range(B):
        xt[("q", b)] = np.ascontiguousarray(query[b].T.astype(bf))
        xt[("k", b)] = np.ascontiguousarray(key[b].T.astype(bf))
        xv = value[b].T.astype(bf)  # [D_MODEL, S]
        # batch-major pretile: [128, 4*ET*512], batch b4 block contiguous
        xvt = (
            xv.reshape(ET, 128, 4, 512)
            .transpose(1, 2, 0, 3)  # [128, 4, ET, 512]
            .reshape(128, 4 * ET * 512)
        )
        xt[("v", b)] = np.ascontiguousarray(xvt)

    in_maps = []
    for c in range(N_CORES):
        b = c // 4
        hb = (c % 4) * HPC
        rs = slice(hb * D_K, (hb + HPC) * D_K)
        in_maps.append(
            {
                "xq": xt[("q", b)],
                "xk": xt[("k", b)],
                "xvt": xt[("v", b)],
                "wq": tile_w(w_q[rs, :]),
                "wk": tile_w(w_k[rs, :]),
                "wv": tile_w(w_v[rs, :]),
                "wo": np.ascontiguousarray(w_o[:, rs].T.astype(bf)),
                "maskt": maskt,
                "idbf": idbf,
                "consts": consts,
            }
        )
    return in_maps


def kernel(query, key, value, mask, w_q, w_k, w_v, w_o):
    from concourse.bass_utils import run_bass_kernel_spmd

    in_maps = _host_prep(query, key, value, mask, w_q, w_k, w_v, w_o)
    nc = _get_program()
    res = run_bass_kernel_spmd(nc, in_maps, list(range(N_CORES)))
    out = np.zeros((B, S, D_MODEL), dtype=np.float32)
    for c in range(N_CORES):
        out[c // 4] += np.asarray(res.results[c]["y"], dtype=np.float32)
    return out


# revision 58
# speedup vs baseline: 1.1281x; 1.0428x over previous
"""Multi-head causal attention (B=2, S=2048, D=1024, H=16) on 8 TRN2 NeuronCores.

Sharding: batch*head parallel. Core c handles batch b = c//4 and the 4
heads h in [4*(c%4), 4*(c%4)+4). Each core computes its heads' Q/K/V
projections (column-parallel), causal softmax attention, and its partial
row-parallel output projection; the host sums the 4 partial outputs per
batch (the AllReduce of row-parallel tensor parallelism).

On-device layout: everything is kept "transposed" (feature-major) so
every matmul contracts along the partition dimension:
  scoresT[k,q] = K Q^T      (per head, 128-row k-tiles x 512-col q-tiles)
  P^T = exp(scoresT/8 + mask/8)   (additive -1e9 causal mask)
  outT[d,q]   = sum_k V[k,d] P^T[k,q]   (PSUM-accumulated over k-tiles)
  sums[q]     = sum_k P^T[k,q]          (ones-vector matmul, col-packed)
  y[q,e]     += sum_hd outT_norm[hd,q] * w_oT[hd,e]
Softmax skips the max-subtraction: scores ~ N(0,1) so exp never
overflows, and exp(-1e9/8) underflows to exactly 0 like the
reference's masked_fill(-1e9).

Schedule (v2, rebuilt from the measured baseline trace):
- All DRAM operands are HOST-pretiled so every input DMA moves >=4KB
  contiguous lines (engine descriptor-gen cost ~0.65us instead of
  2-7us for strided patterns).
- DMA priority order: wq, xq, wk, xk (the 9MB that gates the attention
  stream), then maskt/idbf/wv/xvt/consts/wo, round-robin over the
  sync/gpsimd/scalar queues. The Scalar queue carries only early
  ungated loads so nothing blocks the exp stream.
- Front: warm-up matmuls (PE clock-gate ramp), then Q projection
  (m0+m1 per e-tile, DMA-paced, 8 PSUM banks), drains chunked per
  512 cols across Vector/Scalar/GpSimd, then K projection the same
  way reusing the banks. First scores fire ~0.6us after the first
  K-m0 drain chunk.
- Stream: flat software-pipelined attention as in the baseline, with
  passes ALTERNATING head-pairs ((0,0),(1,0),(0,1),(1,1),...) so the
  row-parallel output projection (needs both pairs per q-block)
  spreads through the stream instead of piling at the end.
- ob accumulator drains ride GpSimd (idle during the stream); psy
  casts ride Vector; y DMAs GpSimd (tail ones alternate with Scalar,
  which is idle after the last exp).
- Tail: the last pass's normalize is interleaved per-128-col chunk
  with its output-projection matmuls, casts and DMAs so the post-exp
  tail is a pipeline, not a serial chain.
"""

import numpy as np

D_MODEL = 1024
N_HEADS = 16
D_K = 64
B, S = 2, 2048
N_CORES = 8
HPC = 4            # heads per core
KT = S // 128      # 16 k-tiles
QT = S // 512      # 4 q-tiles
ET = D_MODEL // 128  # 8 e-tiles (contraction tiles for projections)

_PROG_CACHE = {}


def _build_program():
    import concourse.bacc as bacc_mod
    import concourse.mybir as mybir
    import concourse.tile as tile

    f32 = mybir.dt.float32
    f32r = mybir.dt.float32r
    bf16 = mybir.dt.bfloat16
    Exp = mybir.ActivationFunctionType.Exp

    nc = bacc_mod.Bacc(
        "TRN2", target_bir_lowering=False, debug=False, num_devices=N_CORES
    )

    xq = nc.dram_tensor("xq", [D_MODEL, S], bf16, kind="ExternalInput").ap()
    xk = nc.dram_tensor("xk", [D_MODEL, S], bf16, kind="ExternalInput").ap()
    xvt = nc.dram_tensor("xvt", [128, 4 * ET * 512], bf16, kind="ExternalInput").ap()
    wq = nc.dram_tensor("wq", [128, ET * 256], bf16, kind="ExternalInput").ap()
    wk = nc.dram_tensor("wk", [128, ET * 256], bf16, kind="ExternalInput").ap()
    wv = nc.dram_tensor("wv", [128, ET * 256], bf16, kind="ExternalInput").ap()
    wo = nc.dram_tensor("wo", [256, D_MODEL], bf16, kind="ExternalInput").ap()
    maskt = nc.dram_tensor("maskt", [128, 1024], bf16, kind="ExternalInput").ap()
    idbf = nc.dram_tensor("idbf", [128, 132], bf16, kind="ExternalInput").ap()
    consts = nc.dram_tensor("consts", [128, 193], f32r, kind="ExternalInput").ap()
    y = nc.dram_tensor("y", [S, D_MODEL], bf16, kind="ExternalOutput").ap()

    with (
        tile.TileContext(nc) as tc,
        nc.allow_low_precision("bf16 attention"),
        tc.tile_pool(name="persist", bufs=1) as pp,
    ):
        # ---- persistent SBUF tiles ----
        def persist(shape, dtype, name):
            return pp.tile(shape, dtype, name=name, tag=name)

        wq_sb = persist([128, ET * 256], bf16, "wq_sb")
        wk_sb = persist([128, ET * 256], bf16, "wk_sb")
        wv_sb = persist([128, ET * 256], bf16, "wv_sb")
        wo_sb = [persist([128, D_MODEL], bf16, f"wo_sb{p}") for p in range(2)]
        maskt_sb = persist([128, 1024], bf16, "maskt_sb")
        idbf_sb = persist([128, 132], bf16, "idbf_sb")
        consts_sb = persist([128, 193], f32r, "consts_sb")
        qt_sb = [persist([128, S], bf16, f"qt_sb{p}") for p in range(2)]
        kt_sb = [persist([128, S], bf16, f"kt_sb{p}") for p in range(2)]
        # 128 cols per head (64 V + 1 ones + 63 zeros): the attnV
        # stationary is then exactly 128 weight-rows (FWL trigger).
        v_sb = [persist([128, 512], bf16, f"v_sb{i}") for i in range(KT)]
        outt_sb = [persist([128, S], bf16, f"outt_sb{p}") for p in range(2)]

        # ---- PE warm-up ----
        # The PE HAM clock gate drops to K=4 half-clock after any multi-us
        # PE idle and needs ~3.4us of gapless activity to return to K=8.
        # Dummy matmuls on a memset tile cover the ramp until the first
        # projection e-tile lands (~9.5us: preamble ~7us + wq + xq chunk).
        warm_src = persist([128, 640], bf16, "warm_src")
        nc.vector.memset(warm_src[:], 0.0)
        for i in range(KT):
            nc.vector.memset(v_sb[i][:], 0.0)
        with tc.tile_pool(name="psW", bufs=1, space="PSUM") as psW:
            wt = psW.tile([128, 512], f32, name="warm", tag="warm")
            for w in range(22):
                nc.tensor.matmul(
                    wt[:], warm_src[:, 0:128], warm_src[:, 128:640],
                    start=True, stop=True,
                )

        # ---- DMA issue: everything up front, in priority order ----
        # Three queues (sync HWDGE, gpsimd SWDGE, scalar HWDGE) round-
        # robin.  The stream gate is wq+xq+wk+xk (9MB ~ 25us at HBM BW);
        # everything else follows.
        # all 16 x e-tiles live at once: no WAR gating on any input DMA
        # issue, so the three queues stream the 9MB gate continuously
        xvkp_ctx = tc.tile_pool(name="xvk", bufs=4)
        xvkp = xvkp_ctx.__enter__()
        xep_ctx = tc.tile_pool(name="xe", bufs=16, side="right")
        xep = xep_ctx.__enter__()

        SYNC, GP, SC = nc.sync, nc.gpsimd, nc.scalar
        vdma_tiles = [
            xvkp.tile([128, ET * 512], bf16, name=f"xvk_{b}", tag="xvk")
            for b in range(4)
        ]
        xe_tiles = {}
        for ti in range(2):
            for e in range(ET):
                xe_tiles[(ti, e)] = xep.tile(
                    [128, S], bf16, name=f"xe_{ti}_{e}", tag="xe"
                )

        # The tile scheduler reorders same-engine DMA issues; wait-hints
        # (scheduler-time lower bounds) keep everything that is not the
        # stream gate (wq+xq+wk+xk) behind it in HBM-bandwidth order.
        # wq + the first e-tile ride Sync (HW-DGE) back-to-back so the
        # first projection matmul can start ~11us.
        SYNC.dma_start(out=wq_sb[:], in_=wq[:])
        # first xq e-tile in 4 chunks so the first matmul starts ASAP
        for c in range(4):
            (GP, SC, SYNC, GP)[c].dma_start(
                out=xe_tiles[(0, 0)][:, c * 512 : (c + 1) * 512],
                in_=xq[0:128, c * 512 : (c + 1) * 512],
            )
        xq_engs = (SC, SYNC, GP, SC, SYNC, GP, SC)
        for e in range(1, ET):
            xq_engs[e - 1].dma_start(
                out=xe_tiles[(0, e)][:], in_=xq[e * 128 : (e + 1) * 128, :]
            )
        SYNC.dma_start(out=wk_sb[:], in_=wk[:])
        xk_engs = (SC, SYNC, SC, GP, SYNC, SC, SYNC, GP)
        for e in range(ET):
            xk_engs[e].dma_start(
                out=xe_tiles[(1, e)][:], in_=xk[e * 128 : (e + 1) * 128, :]
            )
        # wait-hints (scheduler-time lower bounds) keep everything that
        # is not the stream gate behind it in HBM-bandwidth order
        with tc.tile_wait_until(0.026):
            GP.dma_start(out=maskt_sb[:], in_=maskt[:])
            GP.dma_start(out=idbf_sb[:], in_=idbf[:])
        with tc.tile_wait_until(0.030):
            SYNC.dma_start(out=wv_sb[:], in_=wv[:])
        for b in range(4):
            with tc.tile_wait_until(0.033 + 0.004 * b):
                (GP, SYNC, GP, SYNC)[b].dma_start(
                    out=vdma_tiles[b][:],
                    in_=xvt[:, b * 4096 : (b + 1) * 4096],
                )
        with tc.tile_wait_until(0.050):
            GP.dma_start(out=consts_sb[:], in_=consts[:])
            for p in range(2):
                (SYNC, GP)[p].dma_start(
                    out=wo_sb[p][:], in_=wo[p * 128 : (p + 1) * 128, :]
                )

        # ---- Q then K projection, e-tile paced, full m0+m1 ----
        psA0_ctx = tc.tile_pool(name="psA0", bufs=1, space="PSUM")
        psA0 = psA0_ctx.__enter__()
        psA1_ctx = tc.tile_pool(name="psA1", bufs=1, space="PSUM", side="right")
        psA1 = psA1_ctx.__enter__()

        def proj_mm(ps, w_tile, m, e, xe, n):
            lhsT = w_tile[:, e * 256 + m * 128 : e * 256 + (m + 1) * 128]
            nc.tensor.matmul(
                ps[:, n * 512 : (n + 1) * 512],
                lhsT,
                xe[:, n * 512 : (n + 1) * 512],
                start=(e == 0),
                stop=(e == ET - 1),
            )

        def drain_chunk(eng, dst_t, ps, n):
            if eng is nc.scalar:
                eng.copy(dst_t[:, n * 512 : (n + 1) * 512],
                         ps[:, n * 512 : (n + 1) * 512])
            else:
                eng.tensor_copy(dst_t[:, n * 512 : (n + 1) * 512],
                                ps[:, n * 512 : (n + 1) * 512])

        # PSUM WAR is tracked at tile granularity, so a drain emitted
        # between two matmuls on the SAME psum tile serializes the PE.
        # Hide m0's drains under m1's e7 matmuls (different tile), and
        # m1's under the next phase's first m0 matmuls.
        for ti, (w_tile, dst) in enumerate(
            ((wq_sb, qt_sb), (wk_sb, kt_sb))
        ):
            ps0 = psA0.tile([128, S], f32, name=f"ps_p{ti}_0", tag="projA", bufs=1)
            ps1 = psA1.tile([128, S], f32, name=f"ps_p{ti}_1", tag="projB", bufs=1)
            for e in range(ET):
                xe = xe_tiles[(ti, e)]
                if e < ET - 1:
                    for m, ps in ((0, ps0), (1, ps1)):
                        for n in range(QT):
                            proj_mm(ps, w_tile, m, e, xe, n)
                else:
                    for n in range(QT):
                        proj_mm(ps0, w_tile, 0, e, xe, n)
                    # K drains stay off Scalar so nothing queues ahead
                    # of the first exp
                    d_eng = (
                        (nc.vector, nc.scalar, nc.vector, nc.scalar)
                        if ti == 0
                        else (nc.vector, nc.vector, nc.vector, nc.vector)
                    )
                    for n in range(QT):
                        proj_mm(ps1, w_tile, 1, e, xe, n)
                        drain_chunk(d_eng[n], dst[0], ps0, n)
                    for n in range(QT):
                        drain_chunk(d_eng[n], dst[1], ps1, n)

        # x e-tiles are dead once the projections are emitted; free the
        # pool so the stream pools below can overlay its SBUF.
        xep_ctx.__exit__(None, None, None)

        # hand the Q/K PSUM banks to the stream pools
        psA0_ctx.__exit__(None, None, None)
        psS_ctx = tc.tile_pool(name="psS", bufs=2, space="PSUM")
        psS = psS_ctx.__enter__()
        psA1_ctx.__exit__(None, None, None)
        psO_ctx = tc.tile_pool(name="psO", bufs=1, space="PSUM")
        psO = psO_ctx.__enter__()
        psV_ctx = tc.tile_pool(name="psV", bufs=2, space="PSUM")
        psV = psV_ctx.__enter__()
        psY = None  # opens once psV closes

        etp_ctx = tc.tile_pool(name="et", bufs=6)
        etp = etp_ctx.__enter__()
        obp_ctx = tc.tile_pool(name="ob", bufs=4)
        obp = obp_ctx.__enter__()
        rcp_ctx = tc.tile_pool(name="rcsb", bufs=3)
        rcp = rcp_ctx.__enter__()
        ysb_ctx = tc.tile_pool(name="ysb", bufs=3)
        ysbp = ysb_ctx.__enter__()

        nvp = 0  # V-projection k-tiles emitted

        def emit_vproj_tile():
            nonlocal nvp
            i = nvp
            psv = psV.tile([128, 256], f32, name=f"psv_{i}", tag="v")
            xvk = vdma_tiles[i // 4]
            k0 = (i % 4) * 128
            for e in range(ET):
                nc.tensor.matmul(
                    psv[:],
                    xvk[:, e * 512 + k0 : e * 512 + k0 + 128],
                    wv_sb[:, e * 256 : (e + 1) * 256],
                    start=(e == 0),
                    stop=(e == ET - 1),
                )
            nc.vector.tensor_copy(
                v_sb[i][:].rearrange("p (h c) -> p h c", c=128)[:, :, 0:64],
                psv[:].rearrange("p (h d) -> p h d", d=64),
            )
            nc.vector.tensor_copy(
                v_sb[i][:].rearrange("p (h c) -> p h c", c=128)[:, :, 64:65],
                idbf_sb[:, 128:132].rearrange("p (h c) -> p h c", c=1),
            )
            nvp += 1

        def emit_warm_filler(count):
            # dependency-free dummy matmuls: keep the PE clock up across
            # unavoidable serial waits (final normalize chain)
            wt2 = psS.tile([128, 1024], f32, name="warm2", tag="s")
            for _ in range(count):
                nc.tensor.matmul(
                    wt2[:, 0:256], idbf_sb[:, 0:128], maskt_sb[:, 0:256],
                    start=True, stop=True,
                )

        ysb_tiles = {}

        def emit_outproj_half(m, n, tail=False):
            psy = psY.tile([128, 512], f32, name=f"psy_{m}_{n}", tag="y")
            for p in range(2):
                nc.tensor.matmul(
                    psy[:],
                    outt_sb[p][:, m * 128 : (m + 1) * 128],
                    wo_sb[p][:, n * 512 : (n + 1) * 512],
                    start=(p == 0),
                    stop=(p == 1),
                )
            if n == 0:
                ysb_tiles[m] = ysbp.tile(
                    [128, 1024], bf16, name=f"y_sb_{m}", tag="ysb"
                )
            y_sb = ysb_tiles[m]
            if tail and n == 0:  # split tail casts across Act and Vector
                nc.scalar.copy(y_sb[:, n * 512 : (n + 1) * 512], psy[:])
            else:
                nc.vector.tensor_copy(y_sb[:, n * 512 : (n + 1) * 512], psy[:])
            if n == 1:
                eng = nc.gpsimd if m % 2 else nc.sync
                eng.dma_start(out=y[m * 128 : (m + 1) * 128, :], in_=y_sb[:])

        def emit_normalize(pr, jj, ob):
            # sums live on row 64 of ob for each head's 512-col half.
            ssb = rcp.tile([33, 512], f32, name=f"ssb_{pr}_{jj}", tag="ssb")
            for hh in range(2):
                nc.vector.tensor_copy(
                    ssb[32 * hh : 32 * hh + 1, :],
                    ob[64:65, 512 * hh : 512 * (hh + 1)],
                )
            rc32 = rcp.tile([33, 512], f32, name=f"rc32_{pr}_{jj}", tag="rc32")
            nc.vector.reciprocal_approx_fast(out=rc32[:], in_=ssb[:])
            rc = rcp.tile([33, 512], f32r, name=f"rc_{pr}_{jj}", tag="rc")
            nc.vector.tensor_copy(rc[:], rc32[:])
            bcs = []
            for hh in range(2):
                bc = psY.tile([128, 512], f32, name=f"ps_bc_{pr}_{jj}_{hh}", tag="y")
                nc.tensor.matmul(
                    bc[0:64, :],
                    consts_sb[32 * hh : 32 * hh + 1, 128:192],
                    rc[32 * hh : 32 * hh + 1, :],
                    start=True,
                    stop=True,
                    tile_position=(32 * hh, 0),
                )
                bcs.append(bc)
            for mo in range(4):
                for hh in range(2):
                    nc.vector.tensor_mul(
                        outt_sb[pr][64 * hh : 64 * hh + 64,
                                    jj * 512 + 128 * mo : jj * 512 + 128 * mo + 128],
                        ob[0:64, 512 * hh + 128 * mo : 512 * hh + 128 * mo + 128],
                        bcs[hh][0:64, 128 * mo : 128 * mo + 128],
                    )

        # ---- the flat attention stream ----
        # Alternate head-pairs so outproj block j unlocks after pass 2j+1.
        passes = [(0, 0), (1, 0), (0, 1), (1, 1), (0, 2), (1, 2), (0, 3), (1, 3)]
        tail_sums = rcp.tile([1, 1024], f32, name="tail_sums", tag="tsums")
        tail_rc32 = rcp.tile([1, 1024], f32, name="tail_rc32", tag="trc32")
        tail_rc = rcp.tile([1, 1024], f32r, name="tail_rc", tag="trc")
        steps = []
        for pr, j in passes:
            for i in range(4 * j + 4):
                steps.append((pr, j, i))

        # earliest step at which V-proj k-tile t may be emitted (xv batch
        # t//4 must have landed).  Compressed: psY (outproj PSUM) can only
        # open once psV closes, and the alternating pass order needs
        # outproj from ~step 18.
        vp_sched = {i: i + 2 for i in range(KT)}

        ps_outs = {}   # (pr, j) -> psO tile
        ob_tiles = {}  # (pr, j) -> SBUF drain tile
        pending = []   # emitted exps awaiting their attnV
        norm_q = []    # (ready_step, pr, j)
        outp_q = []    # (ready_step, m, n) output-projection halves
        pop_hold = 0   # extra pop delay right after a drain (psO WAR)

        def emit_attnv(pr, j, i, et, c0, n_i):
            nonlocal pop_hold
            if (pr, j) not in ps_outs:
                ps_outs[(pr, j)] = psO.tile(
                    [128, 1024], f32, name=f"ps_out_{pr}_{j}", tag="o"
                )
            ps_out = ps_outs[(pr, j)]
            for hh in range(2):
                nc.tensor.matmul(
                    ps_out[:, 512 * hh + c0 : 512 * (hh + 1)],
                    v_sb[i][:, (2 * pr + hh) * 128 : (2 * pr + hh + 1) * 128],
                    et[:, 512 * hh + c0 : 512 * (hh + 1)],
                    start=(i == 0),
                    stop=(i == n_i - 1),
                    skip_group_check=True,
                )
            if i == n_i - 1:
                # pass complete: drain the accumulator and queue the
                # (fully deferrable) normalize
                if (pr, j) == passes[-1]:
                    # tail: per-head 512-col chunks so the first bc can
                    # fire ~1.7us after the last attnV; out rows drain on
                    # Scalar (idle after the last exp)
                    for hh in range(2):
                        cs = slice(512 * hh, 512 * (hh + 1))
                        nc.vector.tensor_copy(tail_sums[:, cs], ps_out[64:65, cs])
                        nc.vector.reciprocal_approx_fast(
                            out=tail_rc32[:, cs], in_=tail_sums[:, cs]
                        )
                        nc.vector.tensor_copy(tail_rc[:, cs], tail_rc32[:, cs])
                    ob = obp.tile([64, 1024], f32, name="ob_tail", tag="obt")
                    nc.scalar.copy(ob[:], ps_out[0:64, :])
                else:
                    ob = obp.tile([65, 1024], f32, name=f"ob_{pr}_{j}", tag="ob")
                    nc.vector.tensor_copy(ob[:], ps_out[0:65, :])
                ob_tiles[(pr, j)] = ob
                pop_hold = 1

        for sidx, (pr, j, i) in enumerate(steps):
            n_i = 4 * j + 4
            # scores (+ causal mask straddle) and exp
            diag = i >= 4 * j
            r = i - 4 * j
            c0 = 128 * r if diag else 0
            qs = slice(j * 512, (j + 1) * 512)
            pss = psS.tile([128, 1024], f32, name=f"ps_s{pr}_{j}_{i}", tag="s")
            if diag:
                for hh in range(2):
                    nc.tensor.matmul(
                        pss[:, 512 * hh + c0 : 512 * hh + c0 + 128],
                        idbf_sb[:, 0:128],
                        maskt_sb[:, r * 256 : r * 256 + 128],
                        start=True,
                        stop=False,
                    )
            for hh in range(2):
                hp = slice(64 * hh, 64 * hh + 64)
                nc.tensor.matmul(
                    pss[:, 512 * hh + c0 : 512 * (hh + 1)],
                    kt_sb[pr][hp, i * 128 : (i + 1) * 128],
                    qt_sb[pr][hp, qs.start + c0 : qs.stop],
                    start=not diag,
                    stop=True,
                    skip_group_check=diag,
                )
            et = etp.tile([128, 1024], bf16, name=f"et{pr}_{j}_{i}", tag="et")
            nc.scalar.activation(et[:, c0:1024], pss[:, c0:1024], Exp, scale=0.125)
            pending.append((pr, j, i, et, c0, n_i))
            if nvp < KT and vp_sched[nvp] <= sidx:
                emit_vproj_tile()
            if len(pending) >= 3 + pop_hold:
                emit_attnv(*pending.pop(0))
                if len(pending) >= 4:  # catch up after a delayed start
                    emit_attnv(*pending.pop(0))
            elif pop_hold:
                pop_hold = 0
            if i == n_i - 1:
                norm_q.append((sidx + 3, pr, j))

            # PE filler after this step's main work
            if (
                norm_q
                and norm_q[0][0] <= sidx
                and psY is not None
                and tuple(norm_q[0][1:]) in ob_tiles
            ):
                _, npr, nj = norm_q.pop(0)
                emit_normalize(npr, nj, ob_tiles.pop((npr, nj)))
                if npr == 1:
                    for mo in range(4):
                        for n in range(2):
                            outp_q.append((sidx + 2 + mo, 4 * nj + mo, n))
            if outp_q and outp_q[0][0] <= sidx and psY is not None:
                _, m, n = outp_q.pop(0)
                emit_outproj_half(m, n)
                # one more half if backlogged
                if outp_q and outp_q[0][0] + 2 <= sidx:
                    _, m, n = outp_q.pop(0)
                    emit_outproj_half(m, n)
            if nvp == KT and psY is None:
                psV_ctx.__exit__(None, None, None)
                psY_ctx = tc.tile_pool(name="psY", bufs=2, space="PSUM")
                psY = psY_ctx.__enter__()

        # tail: flush remaining attnVs, then a pipelined normalize +
        # outproj chain, with warm fillers covering the serial prefix so
        # the PE clock gate never drops.
        while pending:
            emit_attnv(*pending.pop(0))
        emit_warm_filler(28)
        # scores, attnV and mid-stream outproj PSUM are all done: hand
        # the banks to a wide tail ring so the last outproj burst (2 bc
        # tiles + 8 psy halves) never blocks on a bank WAR
        psY_ctx.__exit__(None, None, None)
        psO_ctx.__exit__(None, None, None)
        psS_ctx.__exit__(None, None, None)
        psT_ctx = tc.tile_pool(name="psT", bufs=8, space="PSUM")
        psY = psT_ctx.__enter__()
        while outp_q:
            _, m, n = outp_q.pop(0)
            emit_outproj_half(m, n)
        while norm_q:
            _, npr, nj = norm_q.pop(0)
            if (npr, nj) == passes[-1]:
                # the recip chain already ran (chunked) off the last
                # attnV; bc from partition 0; each 128-col chunk chased
                # by its outproj
                ob = ob_tiles.pop((npr, nj))
                bcs = []
                for hh in range(2):
                    bc = psY.tile([128, 512], f32, name=f"t_bc_{hh}", tag="y")
                    nc.tensor.matmul(
                        bc[0:64, :],
                        consts_sb[0:1, 128:192],
                        tail_rc[0:1, 512 * hh : 512 * (hh + 1)],
                        start=True,
                        stop=True,
                    )
                    bcs.append(bc)
                for mo in range(4):
                    for hh in range(2):
                        nc.vector.tensor_mul(
                            outt_sb[npr][64 * hh : 64 * hh + 64,
                                         nj * 512 + 128 * mo : nj * 512 + 128 * mo + 128],
                            ob[0:64, 512 * hh + 128 * mo : 512 * hh + 128 * mo + 128],
                            bcs[hh][0:64, 128 * mo : 128 * mo + 128],
                        )
                    emit_outproj_half(4 * nj + mo, 0, tail=True)
                    emit_outproj_half(4 * nj + mo, 1, tail=True)
            else:
                emit_normalize(npr, nj, ob_tiles.pop((npr, nj)))
                if npr == 1:
                    for mo in range(4):
                        emit_outproj_half(4 * nj + mo, 0)
                        emit_outproj_half(4 * nj + mo, 1)

        for ctx in (ysb_ctx, rcp_ctx, obp_ctx, etp_ctx):
            ctx.__exit__(None, None, None)
        psT_ctx.__exit__(None, None, None)
        xep_ctx.__exit__(None, None, None)
        xvkp_ctx.__exit__(None, None, None)

    nc.compile()
    return nc


def _get_program():
    if "nc" not in _PROG_CACHE:
        _PROG_CACHE["nc"] = _build_program()
    return _PROG_CACHE["nc"]


def _host_prep(query, key, value, mask, w_q, w_k, w_v, w_o):
    import ml_dtypes

    bf = ml_dtypes.bfloat16
    query = np.asarray(query, dtype=np.float32)
    key = np.asarray(key, dtype=np.float32)
    value = np.asarray(value, dtype=np.float32)
    w_q = np.asarray(w_q, dtype=np.float32)
    w_k = np.asarray(w_k, dtype=np.float32)
    w_v = np.asarray(w_v, dtype=np.float32)
    w_o = np.asarray(w_o, dtype=np.float32)
    m = np.asarray(mask).reshape(S, S).astype(bool)

    # The kernel's block-skip structure assumes the standard causal mask.
    expected = np.triu(np.ones((S, S), dtype=bool), k=1)
    if not np.array_equal(m, expected):
        raise NotImplementedError("kernel specialized for causal (triu, k=1) mask")

    # 4 canonical diagonal-straddle mask tiles: pattern r covers k-tile
    # 4j+r vs q-tile j; masked where (128r + row) > col.
    maskt = np.zeros((128, 1024), dtype=np.float32)
    rows = np.arange(128)[:, None]
    cols = np.arange(128)[None, :]
    straddle = np.where(rows > cols, np.float32(-1e9), np.float32(0.0))
    for r in range(4):
        maskt[:, r * 256 : r * 256 + 128] = straddle
        maskt[:, r * 256 + 128 : r * 256 + 256] = straddle
    maskt = maskt.astype(bf)
    idbf = np.zeros((128, 132), dtype=bf)
    idbf[:, 0:128] = np.eye(128, dtype=bf)
    idbf[:, 128:132] = bf(1.0)

    consts = np.zeros((128, 193), dtype=np.float32)
    consts[:, 0:128] = np.eye(128, dtype=np.float32)
    consts[:, 128:193] = 1.0

    def tile_w(w_rows):  # [256, D_MODEL] -> [128, ET*256] (p (t d))
        return np.ascontiguousarray(
            w_rows.T.reshape(ET, 128, 256).transpose(1, 0, 2).reshape(128, ET * 256)
        ).astype(bf)

    xt = {}
    for b in range(B):
        xt[("q", b)] = np.ascontiguousarray(query[b].T.astype(bf))
        xt[("k", b)] = np.ascontiguousarray(key[b].T.astype(bf))
        xv = value[b].T.astype(bf)  # [D_MODEL, S]
        # batch-major pretile: [128, 4*ET*512], batch b4 block contiguous
        xvt = (
            xv.reshape(ET, 128, 4, 512)
            .transpose(1, 2, 0, 3)  # [128, 4, ET, 512]
            .reshape(128, 4 * ET * 512)
        )
        xt[("v", b)] = np.ascontiguousarray(xvt)

    in_maps = []
    for c in range(N_CORES):
        b = c // 4
        hb = (c % 4) * HPC
        rs = slice(hb * D_K, (hb + HPC) * D_K)
        in_maps.append(
            {
                "xq": xt[("q", b)],
                "xk": xt[("k", b)],
                "xvt": xt[("v", b)],
                "wq": tile_w(w_q[rs, :]),
                "wk": tile_w(w_k[rs, :]),
                "wv": tile_w(w_v[rs, :]),
                "wo": np.ascontiguousarray(w_o[:, rs].T.astype(bf)),
                "maskt": maskt,
                "idbf": idbf,
                "consts": consts,
            }
        )
    return in_maps


def kernel(query, key, value, mask, w_q, w_k, w_v, w_o):
    from concourse.bass_utils import run_bass_kernel_spmd

    in_maps = _host_prep(query, key, value, mask, w_q, w_k, w_v, w_o)
    nc = _get_program()
    res = run_bass_kernel_spmd(nc, in_maps, list(range(N_CORES)))
    out = np.zeros((B, S, D_MODEL), dtype=np.float32)
    for c in range(N_CORES):
        out[c // 4] += np.asarray(res.results[c]["y"], dtype=np.float32)
    return out
